# revision 1
# baseline (speedup 1.0000x reference)
"""Trainium2 Bass kernel for nn_MDCN (mixture-density head forward pass).

Reference computation (B=2048, F=1024, M=128):
    rho = tanh(feature @ h2rho_w.T + h2rho_b);  rho[:, 0] = 0.95
    pi  = softmax(feature @ h2pi_w.T + h2pi_b)
    var0 = exp(feature @ h2var_w.T + h2var_b)
    var = (1 - exp(rho)) * var0 + 1e-4
    W_ = r*muW + s*(r*(zstd/wstd)*(W-muW) + Z*s),  s = sqrt(1-r^2)
    mu = einsum('bmf,bf->bm', W_, feature)

Key algebraic collapse: with a = (zstd/wstd)*(W-muW),
    mu[b,m] = r*d1[b] + r*s*d2[b] + s^2*d3[b]
  where d1 = feature@muW, d2 = feature@a, d3 = feature@Z.
So the [B,M,F] einsum becomes 3 extra columns of one fused matmul:
    logits[b, 0:387] = feature[b] @ [wrho.T | wpi.T | wvar.T | muW | a | Z]
Additionally s = sqrt(1-r^2) = sech(u) = (1+tanh(u)) * exp(-u), so the whole
epilogue needs only Tanh and Exp (one ACT table set), and the clamped first
mixture column is a compile-time constant.

Sharding: pure data-parallel over batch across 8 cores (256 rows/core),
weights replicated. No collectives needed (forward only).
"""

import os
from contextlib import ExitStack

import numpy as np

import concourse.bass as bass
import concourse.bacc as bacc
import concourse.mybir as mybir
import concourse.tile as tile
from concourse.bass_utils import run_bass_kernel_spmd

B, F, M = 2048, 1024, 128
NCORES = 8
BC = B // NCORES            # 256 batch rows per core
NT = BC // 128              # 2 partition tiles per core
KC = F // 128               # 8 contraction chunks
NW = 3 * M + 4              # 388 fused output columns (384 logits +
                            # 3 mu dot-products + 1 pad; fp32r matmul
                            # requires an even destination free-dim)
RHO_1 = np.float32(0.95)
TAU_INV = 1.0e-4
# s at the clamped column, computed exactly as the fp32 reference does:
# s0 = sqrt(1 - 0.95f * 0.95f)
S0 = float(np.sqrt(np.float32(1.0) - RHO_1 * RHO_1))

F32 = mybir.dt.float32
F32R = mybir.dt.float32r
F16 = mybir.dt.float16
AF = mybir.ActivationFunctionType
OP = mybir.AluOpType

# Matmul operand dtype. The per-core DMA path sustains only ~200 GB/s, so
# the kernel is input-bandwidth-bound and fp16 inputs halve its runtime.
# fp16 (11-bit mantissa) keeps the worst-case output error ~1e-3 of scale
# (vs 2.8e-4 for float32r, 4e-3 for bfloat16); accumulation is fp32 in PSUM.
# Set to F32R for a full-precision fallback (bit-compatible with fp32 but
# streams 1 row/cycle vs 4 for plain fp32).
MM_DT = F16
MM_NP = np.float16 if MM_DT == F16 else np.float32


def _emit_body(nc, tc, pools, fwc_dram, ft1_dram, blk_dram, out_dram):
    """Emit one full forward pass: DMA in -> fused matmul -> epilogue -> out."""
    consts, fwpool, psum, work = pools

    # Tiny bias block goes on the gpsimd (SWDGE) queue so it does not block
    # the head of the SP (HWDGE) queue that streams the big inputs.
    blk = consts.tile([1, 128 + NW], MM_DT, tag="bias_blk", name="bias_blk")
    nc.gpsimd.dma_start(blk[:], blk_dram)

    # Each dma_start carries ~0.5-2us of fixed cost on this part, so inputs
    # are consolidated into three transfers: two fused feature+weight halves
    # (PE starts on the first while the second streams) and tile-1 features.
    H = KC // 2
    fwcA = fwpool.tile([128, H, 128 + NW], MM_DT, tag="fwcA", name="fwcA")
    nc.sync.dma_start(fwcA[:], fwc_dram[0:H].rearrange("c p j -> p c j"))
    fwcB = fwpool.tile([128, H, 128 + NW], MM_DT, tag="fwcB", name="fwcB")
    nc.sync.dma_start(fwcB[:], fwc_dram[H:KC].rearrange("c p j -> p c j"))
    ft1 = fwpool.tile([128, KC, 128], MM_DT, tag="ft1", name="ft1")
    nc.sync.dma_start(ft1[:], ft1_dram)

    def fwc_c(c):
        return fwcA[:, c, :] if c < H else fwcB[:, c - H, :]

    # Fused matmul: psum[t][b, :] = bias + sum_c featT_c[:,b].T @ wcat_c
    pt = [psum.tile([128, NW], F32, tag=f"psum{t}", name=f"psum{t}")
          for t in range(NT)]
    for t in range(NT):
        nc.tensor.matmul(pt[t][:], blk[:, 0:128], blk[:, 128:128 + NW],
                         start=True, stop=False)
    for c in range(KC):
        nc.tensor.matmul(pt[0][:], fwc_c(c)[:, 0:128],
                         fwc_c(c)[:, 128:128 + NW],
                         start=False, stop=(c == KC - 1))
    for c in range(KC):
        nc.tensor.matmul(pt[1][:], ft1[:, c, :],
                         fwc_c(c)[:, 128:128 + NW],
                         start=False, stop=(c == KC - 1))

    # Epilogue per 128-row tile. Layout of psum P: [rho | pi | var | d1 d2 d3 0]
    for t in range(NT):
        P = pt[t][:]
        tg = f"t{t}"

        # psum layout (rho weights negated on host): [-u | pi | var | d 0].
        # r = tanh(u) = tanh(-1 * P[:,0:M]); one exp covers e^-u, e^pi, e^var.
        r = work.tile([128, M], F32, tag="r" + tg, name="r" + tg)
        nc.scalar.activation(r[:], P[:, 0:M], AF.Tanh, scale=-1.0)
        E = work.tile([128, 3 * M], F32, tag="E" + tg, name="E" + tg)
        nc.scalar.activation(E[:], P[:, 0:3 * M], AF.Exp)
        eneg, epi, var0 = E[:, 0:M], E[:, M:2 * M], E[:, 2 * M:3 * M]

        dsb = work.tile([128, 3], F32, tag="dsb" + tg, name="dsb" + tg)
        nc.vector.tensor_copy(dsb[:], P[:, 3 * M:3 * M + 3])

        # clamp first mixture BEFORE exp(rho) and the mu chain
        nc.vector.memset(r[:, 0:1], float(RHO_1))
        erho = work.tile([128, M], F32, tag="erho" + tg, name="erho" + tg)
        nc.scalar.activation(erho[:], r[:], AF.Exp)

        out_sb = work.tile([128, 3 * M], F32, tag="out" + tg, name="out" + tg)

        # s = (1 + r) * exp(-u) = sqrt(1 - r^2); fix clamped column
        s = work.tile([128, M], F32, tag="s" + tg, name="s" + tg)
        nc.vector.scalar_tensor_tensor(s[:], r[:], 1.0, eneg, OP.add, OP.mult)
        nc.vector.memset(s[:, 0:1], S0)

        # mu = r*(d1 + s*d2) + s^2*d3
        ss = work.tile([128, M], F32, tag="ss" + tg, name="ss" + tg)
        nc.vector.tensor_mul(ss[:], s[:], s[:])
        q = work.tile([128, M], F32, tag="q" + tg, name="q" + tg)
        nc.scalar.activation(q[:], s[:], AF.Identity,
                             bias=dsb[:, 0:1], scale=dsb[:, 1:2])
        rq = work.tile([128, M], F32, tag="rq" + tg, name="rq" + tg)
        nc.vector.tensor_mul(rq[:], r[:], q[:])
        nc.vector.scalar_tensor_tensor(out_sb[:, M:2 * M], ss[:], dsb[:, 2:3],
                                       rq[:], OP.mult, OP.add)

        # var = (1 - erho) * var0 + tau = -((erho - 1) * var0) + tau
        t1 = work.tile([128, M], F32, tag="t1" + tg, name="t1" + tg)
        nc.vector.scalar_tensor_tensor(t1[:], erho[:], 1.0, var0, OP.subtract,
                                       OP.mult)
        nc.vector.tensor_scalar(out_sb[:, 2 * M:3 * M], t1[:], -1.0, TAU_INV,
                                OP.mult, OP.add)

        # pi = epi / sum(epi)
        ssum = work.tile([128, 1], F32, tag="ssum" + tg, name="ssum" + tg)
        nc.vector.tensor_reduce(ssum[:], epi, mybir.AxisListType.X, OP.add)
        rsum = work.tile([128, 1], F32, tag="rsum" + tg, name="rsum" + tg)
        nc.vector.reciprocal(rsum[:], ssum[:])
        nc.vector.tensor_scalar_mul(out_sb[:, 0:M], epi, rsum[:])

        nc.sync.dma_start(out_dram[t * 128:(t + 1) * 128, :], out_sb[:])


def _declare_io(nc):
    # fwc: per contraction chunk, batch-tile-0 features fused with the weight
    # block (one DMA -> one matmul wait, and tile 0's inputs finish ~0.5MB of
    # DMA earlier than tile 1's, so its epilogue overlaps tile 1's loads).
    # ft1: tile-1 features, loaded last as two contiguous-run DMAs.
    fwc_dram = nc.dram_tensor("fwc", [KC, 128, 128 + NW], MM_DT,
                              kind="ExternalInput").ap()
    ft1_dram = nc.dram_tensor("ft1", [128, KC, 128], MM_DT,
                              kind="ExternalInput").ap()
    blk_dram = nc.dram_tensor("bias_blk", [1, 128 + NW], MM_DT,
                              kind="ExternalInput").ap()
    out_dram = nc.dram_tensor("out", [BC, 3 * M], F32, kind="ExternalOutput").ap()
    return fwc_dram, ft1_dram, blk_dram, out_dram


def _warmup_act(nc, consts):
    # Trigger the ACT exp/tanh table load immediately, overlapping the
    # input DMAs (it costs ~2.7us once per kernel).
    warm_in = consts.tile([128, 1], F32, tag="warm_in", name="warm_in")
    warm_out = consts.tile([128, 1], F32, tag="warm_out", name="warm_out")
    nc.vector.memset(warm_in[:], 0.0)
    nc.scalar.activation(warm_out[:], warm_in[:], AF.Exp)


def _warmup_pe(nc, consts, psum, n_fillers=9):
    # The PE HAM clock-gate only unthrottles (1.2 -> 2.4 GHz) after ~3.4us of
    # sustained activity. Feed it scratch matmuls while the input DMAs stream
    # so the real (dependency-gated) matmuls run at full clock.
    wsrc = consts.tile([1, 128], MM_DT, tag="pe_w", name="pe_w")
    nc.vector.memset(wsrc[:], 1.0)
    msrc = consts.tile([1, 512], MM_DT, tag="pe_m", name="pe_m")
    nc.vector.memset(msrc[:], 1.0)
    scratch = psum.tile([128, 512], F32, tag="pe_scratch", name="pe_scratch",
                        bufs=1)
    for i in range(n_fillers):
        nc.tensor.matmul(scratch[:], wsrc[:], msrc[:], start=True, stop=True)


def _build_nc():
    nc = bacc.Bacc("TRN2", target_bir_lowering=False, debug=False)
    fwc_dram, ft1_dram, blk_dram, out_dram = _declare_io(nc)
    with tile.TileContext(nc) as tc, ExitStack() as ctx:
        consts = ctx.enter_context(tc.tile_pool(name="consts", bufs=1))
        fwpool = ctx.enter_context(tc.tile_pool(name="fw", bufs=1))
        psum = ctx.enter_context(tc.tile_pool(name="psum", bufs=NT, space="PSUM"))
        work = ctx.enter_context(tc.tile_pool(name="work", bufs=NT))
        _warmup_act(nc, consts)
        _warmup_pe(nc, consts, psum)
        _emit_body(nc, tc, (consts, fwpool, psum, work),
                   fwc_dram, ft1_dram, blk_dram, out_dram)
    nc.compile()
    return nc


def build_loop_nc(reps):
    """Timing variant: run the body `reps` times inside one NEFF (used only
    by the local test harness; the default full-barrier back-edge keeps
    iterations serialized so per-iter span ~ single-shot kernel time)."""
    nc = bacc.Bacc("TRN2", target_bir_lowering=False, debug=False)
    fwc_dram, ft1_dram, blk_dram, out_dram = _declare_io(nc)
    with tile.TileContext(nc) as tc, ExitStack() as ctx:
        consts = ctx.enter_context(tc.tile_pool(name="consts", bufs=1))
        fwpool = ctx.enter_context(tc.tile_pool(name="fw", bufs=1))
        psum = ctx.enter_context(tc.tile_pool(name="psum", bufs=NT, space="PSUM"))
        work = ctx.enter_context(tc.tile_pool(name="work", bufs=NT))
        _warmup_act(nc, consts)
        with tc.For_i(0, reps, 1):
            _warmup_pe(nc, consts, psum)
            _emit_body(nc, tc, (consts, fwpool, psum, work),
                       fwc_dram, ft1_dram, blk_dram, out_dram)
    nc.compile()
    return nc


_CACHE = {}


def _get_nc():
    if "nc" not in _CACHE:
        _CACHE["nc"] = _build_nc()
    return _CACHE["nc"]


def _host_prep(inputs):
    f32 = np.float32
    feature = np.ascontiguousarray(inputs["feature"], dtype=f32)
    muW = np.asarray(inputs["muW"], dtype=f32)
    W = np.asarray(inputs["W"], dtype=f32)
    Z = np.asarray(inputs["Z"], dtype=f32)
    logvarW = np.asarray(inputs["logvarW"], dtype=f32)
    logvarZ = np.asarray(inputs["logvarZ"], dtype=f32)

    wstd = np.sqrt(np.exp(logvarW)).astype(f32)
    zstd = np.sqrt(np.exp(logvarZ)).astype(f32)
    a = ((zstd / wstd).astype(f32) * (W - muW)).astype(f32)
    v3 = np.stack([muW, a, Z, np.zeros_like(muW)], axis=1)  # [F, 4]

    wcat = np.concatenate(
        [-np.asarray(inputs["h2rho_w"], dtype=f32).T,
         np.asarray(inputs["h2pi_w"], dtype=f32).T,
         np.asarray(inputs["h2var_w"], dtype=f32).T,
         v3],
        axis=1,
    )  # [F, 387]
    wcat = wcat.reshape(KC, 128, NW)

    bias_blk = np.concatenate(
        [np.ones(128, dtype=f32),
         -np.asarray(inputs["h2rho_b"], dtype=f32),
         np.asarray(inputs["h2pi_b"], dtype=f32),
         np.asarray(inputs["h2var_b"], dtype=f32),
         np.zeros(4, dtype=f32)],
    ).reshape(1, 128 + NW)
    bias_blk = np.ascontiguousarray(bias_blk)

    in_maps = []
    for c in range(NCORES):
        shard = feature[c * BC:(c + 1) * BC]            # [BC, F]
        featT = shard.T.reshape(KC, 128, NT, 128)       # [c, p, half, j]
        fwc = np.ascontiguousarray(
            np.concatenate([featT[:, :, 0, :], wcat], axis=2),
            dtype=MM_NP)                                # [KC,128,128+NW]
        ft1 = np.ascontiguousarray(
            featT[:, :, 1, :].transpose(1, 0, 2), dtype=MM_NP)  # [128(p),KC,128]
        in_maps.append({"fwc": fwc, "ft1": ft1,
                        "bias_blk": bias_blk.astype(MM_NP)})
    return in_maps


def kernel(**inputs):
    nc = _get_nc()
    in_maps = _host_prep(inputs)
    res = run_bass_kernel_spmd(nc, in_maps, list(range(NCORES)))
    full = np.concatenate([res.results[c]["out"] for c in range(NCORES)], axis=0)
    pi = np.ascontiguousarray(full[:, 0:M])
    mu = np.ascontiguousarray(full[:, M:2 * M])
    var = np.ascontiguousarray(full[:, 2 * M:3 * M])
    return pi, mu, var

